# revision 26
# baseline (speedup 1.0000x reference)
# Banked (MoE top-2) feedforward on 8 TRN2 NeuronCores.
#
# Strategy (expert-parallel, per sharding hint):
#   - Router (tiny: [T,1024]@[1024,16]) runs on host with jax-CPU, matching the
#     reference's einsum/softmax/top_k numerics so bank selection is identical.
#   - Host dispatches tokens to banks (the "all-to-all"): per-bank gather of the
#     selected tokens, padded to a per-slot capacity, transposed feature-major.
#   - Each core owns 2 of the 16 banks (slot 0: one of the 8 busiest banks,
#     slot 1: one of the 8 least busy — slot capacities differ, minimizing
#     padding) and runs the two-layer FFN for its banks' tokens:
#     HT = relu(W1^T X^T + b1), YT = W2^T HT + b2, all feature-major so weights
#     are consumed in their natural [K, M] layout as matmul lhsT and no
#     on-device transposes are needed.
#   - Host combines: out[t] = sum_e gate[t,e] * Y_e[t].
#
# Matmul dtype: float16 — full-rate PE with 2-byte FWL weight loads and half
# the weight DMA of fp32, at ~3.5e-4 relative error (values here are far from
# fp16 range limits; accumulation is fp32 in PSUM). BANKED_DTYPE selects
# f32r (~1.6e-4, ~11% slower) / bf16 (~3e-3) / f32 for experiments.

import os
import sys
import types

import numpy as np

# The device kernel runs through jax's axon PJRT backend. If the caller's
# environment pins JAX_PLATFORMS (e.g. to "cpu" for the reference), extend it
# so the axon backend stays reachable. Must happen before jax's first import.
_jp = os.environ.get("JAX_PLATFORMS", "")
if _jp and "axon" not in _jp.split(","):
    os.environ["JAX_PLATFORMS"] = _jp + ",axon"

D_MODEL = 1024
D_HIDDEN = 4096
NUM_BANKS = 16
NUM_SELECTED = 2
N_CORES = 8
P = 128
KD = D_MODEL // P    # 8  contraction subtiles for fc1
MH = D_HIDDEN // P   # 32 hidden subtiles (fc1 out / fc2 contraction)
MO = D_MODEL // P    # 8  output subtiles for fc2

_PROFILE = bool(int(os.environ.get("BANKED_PROFILE", "0")))
_DTYPE = os.environ.get("BANKED_DTYPE", "f16")  # "f16" | "f32r" | "bf16" | "f32"

last_exec_time_ns = None
last_results = None


def _install_ntff_hook():
    """bass_utils' axon trace path imports antenv.axon_hooks, which this image
    lacks; shim it and register the ctypes-based NTFF hook."""
    if "antenv.axon_hooks" in sys.modules:
        return
    mod = types.ModuleType("antenv.axon_hooks")
    mod._hook = None
    mod.set_axon_ntff_profile_hook = lambda h: setattr(mod, "_hook", h)
    mod.get_axon_ntff_profile_hook = lambda: mod._hook
    sys.modules["antenv.axon_hooks"] = mod
    try:
        from trn_agent_boot.trn_boot import _ntff_profile_via_ctypes

        mod.set_axon_ntff_profile_hook(
            _ntff_profile_via_ctypes("/opt/axon/libaxon_pjrt.so")
        )
    except Exception as e:  # profiling is best-effort
        print("ntff hook setup failed:", e)


def _router(tensor_f32, Wr, br):
    """Return (topv, topi) exactly as the reference computes them (jax on CPU)."""
    try:
        import jax
        import jax.numpy as jnp

        cpu = jax.devices("cpu")[0]
        with jax.default_device(cpu):
            t = jax.device_put(jnp.asarray(tensor_f32), cpu)
            w = jax.device_put(jnp.asarray(Wr), cpu)
            b = jax.device_put(jnp.asarray(br), cpu)
            logits = jnp.einsum("bsd,de->bse", t, w) + b
            probs = jax.nn.softmax(logits, axis=-1)
            topv, topi = jax.lax.top_k(probs, NUM_SELECTED)
        return np.asarray(topv), np.asarray(topi)
    except Exception:
        # numpy fallback replicating jax semantics (stable ties by lower index)
        logits = (
            tensor_f32.reshape(-1, D_MODEL) @ np.asarray(Wr, np.float32)
        ) + np.asarray(br, np.float32)
        logits = logits.reshape(tensor_f32.shape[0], tensor_f32.shape[1], NUM_BANKS)
        m = logits.max(axis=-1, keepdims=True)
        e = np.exp(logits - m)
        probs = e / e.sum(axis=-1, keepdims=True)
        order = np.argsort(-probs, axis=-1, kind="stable")
        topi = order[..., :NUM_SELECTED]
        topv = np.take_along_axis(probs, topi, axis=-1)
        return topv.astype(np.float32), topi.astype(np.int32)


def _cap_for(n):
    """Slot capacity for a max bank load of n tokens (even, >= 128)."""
    return max((int(n) + 1) & ~1, P)


def _chunks_for(C):
    """Split C into matmul free-dim chunks <= 512 of near-equal size
    (each >= 256 where possible — float32r needs that for full rate)."""
    out = []
    t0 = 0
    n = (C + 511) // 512
    base = C // n
    rem = C - base * n
    for i in range(n):
        tn = base + (1 if i < rem else 0)
        out.append((t0, tn))
        t0 += tn
    return out


def _build_program(caps, chunks_list, dtype_tag):
    import concourse.mybir as mybir
    import concourse.tile as tile
    from concourse import bacc

    if dtype_tag == "bf16":
        wdt = mybir.dt.bfloat16
    elif dtype_tag == "f16":
        wdt = mybir.dt.float16
    elif dtype_tag == "f32":
        wdt = mybir.dt.float32
    else:
        # float32r end-to-end: walrus requires f32r matmul operands to be
        # produced as f32r, so the dram tensors and sbuf tiles are all f32r
        # (np-side arrays stay float32 — same 4-byte storage).
        wdt = mybir.dt.float32r

    f32 = mybir.dt.float32
    nc = bacc.Bacc("TRN2", target_bir_lowering=False, debug=False, num_devices=N_CORES)

    xts = [
        nc.dram_tensor(f"xt{s}", [KD, P, caps[s]], wdt, kind="ExternalInput").ap()
        for s in range(2)
    ]
    w1 = nc.dram_tensor("w1", [2, MH, P, KD, P], wdt, kind="ExternalInput").ap()
    b1 = nc.dram_tensor("b1", [2, P, MH], f32, kind="ExternalInput").ap()
    w2 = nc.dram_tensor("w2", [2, MO, P, MH, P], wdt, kind="ExternalInput").ap()
    b2 = nc.dram_tensor("b2", [2, P, MO], f32, kind="ExternalInput").ap()
    yts = [
        nc.dram_tensor(f"yt{s}", [MO, P, caps[s]], f32, kind="ExternalOutput").ap()
        for s in range(2)
    ]

    Add = mybir.AluOpType.add
    Max = mybir.AluOpType.max
    tcmax = max(tn for chunks in chunks_list for _, tn in chunks)

    with tile.TileContext(nc) as tc:
        with (
            tc.tile_pool(name="xp", bufs=1) as xp,
            tc.tile_pool(name="bp", bufs=1) as bp,
            tc.tile_pool(name="w1p", bufs=12) as w1p,
            tc.tile_pool(name="w2p", bufs=5) as w2p,
            tc.tile_pool(name="htp", bufs=1) as htp,
            tc.tile_pool(name="yp", bufs=4) as yp,
            tc.tile_pool(name="ps1", bufs=4, space="PSUM") as ps1,
            tc.tile_pool(name="ps2", bufs=4, space="PSUM") as ps2,
        ):
            # Per-(slot, k, chunk) x tiles: contiguous DMAs with fine-grained
            # deps so the first matmuls start as soon as their slice lands.
            # DMA issue is spread over engine queues: w1 on sync, w2 + slot-0 x
            # on scalar, slot-1 x + biases on gpsimd (only SP/ACT/POOL can
            # issue DMAs; gpsimd descriptor issue is the slowest at ~790ns).
            xsb = {}

            def load_x(s, eng):
                # contiguous per-k DMAs with fine-grained deps
                for ci, (t0, tn) in enumerate(chunks_list[s]):
                    for k in range(KD):
                        t = xp.tile(
                            [P, tn], wdt, tag=f"x_{s}_{k}_{ci}", name=f"x_{s}_{k}_{ci}"
                        )
                        eng.dma_start(t[:], xts[s][k, :, t0 : t0 + tn])
                        xsb[(s, k, ci)] = t

            # HAM warm-up: zero matmuls with no DMA deps fill the startup
            # ramp so the PE clock gate opens (1.2 -> 2.4 GHz) before real
            # matmuls arrive.
            wz = bp.tile([P, P], wdt, tag="wz")
            xz = bp.tile([P, tcmax], wdt, tag="xz")
            nc.vector.memset(wz[:], 0.0)
            nc.vector.memset(xz[:], 0.0)
            for i in range(30):
                pz = ps1.tile([P, tcmax], f32, tag="ps1", name=f"pz_{i}")
                nc.tensor.matmul(pz[:], wz[:], xz[:], start=True, stop=True)

            load_x(0, nc.scalar)
            b1sb = bp.tile([P, 2, MH], f32, tag="b1")
            nc.gpsimd.dma_start(b1sb[:], b1.rearrange("e p m -> p e m"))
            b2sb = bp.tile([P, 2, MO], f32, tag="b2")
            nc.gpsimd.dma_start(b2sb[:], b2.rearrange("e p m -> p e m"))

            for s in range(2):
                chunks = chunks_list[s]
                C = caps[s]
                if s == 1:
                    load_x(1, nc.gpsimd)
                ht = htp.tile([P, MH, C], wdt, tag="ht")
                # At kernel start only chunk 0's x is likely resident; emit the
                # first G hm-groups chunk-0-only so the PE has a runway while
                # the rest of x streams in.
                if s == 0 and len(chunks) > 1:
                    G = min(8, MH)
                    group_order = (
                        [(hm, 0) for hm in range(G)]
                        + [(hm, ci) for hm in range(G) for ci in range(1, len(chunks))]
                        + [(hm, ci) for hm in range(G, MH) for ci in range(len(chunks))]
                    )
                else:
                    group_order = [
                        (hm, ci) for hm in range(MH) for ci in range(len(chunks))
                    ]
                w1sbs = {}
                for hm, ci in group_order:
                    if hm not in w1sbs:
                        w1sb = w1p.tile([P, KD, P], wdt, tag="w1")
                        nc.sync.dma_start(w1sb[:], w1[s, hm])
                        w1sbs[hm] = w1sb
                    w1sb = w1sbs[hm]
                    t0, tn = chunks[ci]
                    ps = ps1.tile([P, tcmax], f32, tag="ps1", name=f"ps1_{hm}_{ci}")
                    for k in range(KD):
                        nc.tensor.matmul(
                            ps[:, :tn],
                            w1sb[:, k],
                            xsb[(s, k, ci)][:, :tn],
                            start=(k == 0),
                            stop=(k == KD - 1),
                        )
                    # relu(psum + b1) on DVE (ACT's per-op overhead is huge)
                    nc.vector.tensor_scalar(
                        ht[:, hm, t0 : t0 + tn],
                        ps[:, :tn],
                        b1sb[:, s, hm : hm + 1],
                        0.0,
                        Add,
                        Max,
                    )
                for mo in range(MO):
                    # two half-tiles so the first 16 k2 matmuls can start
                    # after half the weight DMA
                    MHH = MH // 2
                    w2sbs = []
                    for h in range(2):
                        w2sb = w2p.tile([P, MHH, P], wdt, tag="w2", name=f"w2_{h}")
                        nc.scalar.dma_start(
                            w2sb[:], w2[s, mo, :, h * MHH : (h + 1) * MHH]
                        )
                        w2sbs.append(w2sb)
                    pss = [
                        ps2.tile([P, tcmax], f32, tag="ps2", name=f"ps2_{ci}")
                        for ci in range(len(chunks))
                    ]
                    for k2 in range(MH):
                        for ci, (t0, tn) in enumerate(chunks):
                            nc.tensor.matmul(
                                pss[ci][:, :tn],
                                w2sbs[k2 // MHH][:, k2 % MHH],
                                ht[:, k2, t0 : t0 + tn],
                                start=(k2 == 0),
                                stop=(k2 == MH - 1),
                            )
                    for ci, (t0, tn) in enumerate(chunks):
                        ysb = yp.tile([P, tcmax], f32, tag="y")
                        nc.vector.tensor_scalar_add(
                            ysb[:, :tn],
                            pss[ci][:, :tn],
                            b2sb[:, s, mo : mo + 1],
                        )
                        nc.sync.dma_start(yts[s][mo, :, t0 : t0 + tn], ysb[:, :tn])

    nc.compile()
    return nc


def kernel(tensor, Wr, br, W1, b1, W2, b2):
    global last_exec_time_ns, last_results
    from concourse import bass_utils

    t_np = np.asarray(tensor, np.float32)
    B, S, _ = t_np.shape
    T = B * S
    x = np.ascontiguousarray(t_np.reshape(T, D_MODEL))

    topv, topi = _router(t_np, np.asarray(Wr, np.float32), np.asarray(br, np.float32))
    topv = topv.reshape(T, NUM_SELECTED)
    topi = topi.reshape(T, NUM_SELECTED)

    # Per-bank token index lists + gates
    idx = []
    gates = []
    for e in range(NUM_BANKS):
        sel = np.nonzero((topi == e).any(axis=1))[0]
        idx.append(sel)
        g = np.where(topi[sel, 0] == e, topv[sel, 0], topv[sel, 1])
        gates.append(g.astype(np.float32))

    counts = np.array([len(i) for i in idx])
    # Slot 0 hosts the 8 busiest banks, slot 1 the 8 least busy; the two
    # slots get independent capacities, minimizing padded tokens.
    order = np.argsort(-counts, kind="stable")
    slot_banks = [order[:N_CORES], order[N_CORES:]]
    caps = [_cap_for(counts[sb].max()) for sb in slot_banks]
    chunks_list = [_chunks_for(c) for c in caps]

    if _DTYPE == "bf16":
        import ml_dtypes

        np_wdt = ml_dtypes.bfloat16
    elif _DTYPE == "f16":
        np_wdt = np.float16
    else:
        np_wdt = np.float32

    # Feature-major gathered tokens per slot: xt[s][c, k, p, t]
    xt_slot = [np.zeros((N_CORES, KD, P, caps[s]), dtype=np_wdt) for s in range(2)]
    for s in range(2):
        for c in range(N_CORES):
            e = slot_banks[s][c]
            n_e = counts[e]
            if n_e:
                xe_t = x[idx[e]].T.astype(np_wdt)  # [D_MODEL, n_e]
                xt_slot[s][c, :, :, :n_e] = xe_t.reshape(KD, P, n_e)

    W1_np = np.asarray(W1, np.float32)
    W2_np = np.asarray(W2, np.float32)
    w1d = np.ascontiguousarray(
        W1_np.reshape(NUM_BANKS, KD, P, MH, P).transpose(0, 3, 2, 1, 4).astype(np_wdt)
    )
    w2d = np.ascontiguousarray(
        W2_np.reshape(NUM_BANKS, MH, P, MO, P).transpose(0, 3, 2, 1, 4).astype(np_wdt)
    )
    b1d = np.ascontiguousarray(
        np.asarray(b1, np.float32).reshape(NUM_BANKS, MH, P).transpose(0, 2, 1)
    )
    b2d = np.ascontiguousarray(
        np.asarray(b2, np.float32).reshape(NUM_BANKS, MO, P).transpose(0, 2, 1)
    )

    nc = _build_program(caps, chunks_list, _DTYPE)

    in_maps = []
    for c in range(N_CORES):
        pair = [slot_banks[0][c], slot_banks[1][c]]
        in_maps.append(
            {
                "xt0": xt_slot[0][c],
                "xt1": xt_slot[1][c],
                "w1": w1d[pair],
                "b1": b1d[pair],
                "w2": w2d[pair],
                "b2": b2d[pair],
            }
        )

    if _PROFILE:
        _install_ntff_hook()
    res = bass_utils.run_bass_kernel_spmd(
        nc, in_maps, core_ids=list(range(N_CORES)), trace=_PROFILE
    )
    last_exec_time_ns = res.exec_time_ns
    last_results = res

    out = np.zeros((T, D_MODEL), dtype=np.float32)
    for c in range(N_CORES):
        for s in range(2):
            e = slot_banks[s][c]
            n_e = counts[e]
            if n_e:
                ytc = res.results[c][f"yt{s}"]  # [MO, P, caps[s]] f32
                ye = ytc.reshape(D_MODEL, caps[s])[:, :n_e]
                out[idx[e]] += gates[e][:, None] * ye.T
    return out.reshape(B, S, D_MODEL)


# revision 27
# speedup vs baseline: 1.0155x; 1.0155x over previous
# Banked (MoE top-2) feedforward on 8 TRN2 NeuronCores.
#
# Strategy (expert-parallel, per sharding hint):
#   - Router (tiny: [T,1024]@[1024,16]) runs on host with jax-CPU, matching the
#     reference's einsum/softmax/top_k numerics so bank selection is identical.
#   - Host dispatches tokens to banks (the "all-to-all"): per-bank gather of the
#     selected tokens, padded to a per-slot capacity, transposed feature-major.
#   - Each core owns 2 of the 16 banks (slot 0: one of the 8 busiest banks,
#     slot 1: one of the 8 least busy — slot capacities differ, minimizing
#     padding) and runs the two-layer FFN for its banks' tokens:
#     HT = relu(W1^T X^T + b1), YT = W2^T HT + b2, all feature-major so weights
#     are consumed in their natural [K, M] layout as matmul lhsT and no
#     on-device transposes are needed.
#   - Host combines: out[t] = sum_e gate[t,e] * Y_e[t].
#
# Matmul dtype: float16 — full-rate PE with 2-byte FWL weight loads and half
# the weight DMA of fp32, at ~3.5e-4 relative error (values here are far from
# fp16 range limits; accumulation is fp32 in PSUM). BANKED_DTYPE selects
# f32r (~1.6e-4, ~11% slower) / bf16 (~3e-3) / f32 for experiments.

import os
import sys
import types

import numpy as np

# The device kernel runs through jax's axon PJRT backend. If the caller's
# environment pins JAX_PLATFORMS (e.g. to "cpu" for the reference), extend it
# so the axon backend stays reachable. Must happen before jax's first import.
_jp = os.environ.get("JAX_PLATFORMS", "")
if _jp and "axon" not in _jp.split(","):
    os.environ["JAX_PLATFORMS"] = _jp + ",axon"

D_MODEL = 1024
D_HIDDEN = 4096
NUM_BANKS = 16
NUM_SELECTED = 2
N_CORES = 8
P = 128
KD = D_MODEL // P    # 8  contraction subtiles for fc1
MH = D_HIDDEN // P   # 32 hidden subtiles (fc1 out / fc2 contraction)
MO = D_MODEL // P    # 8  output subtiles for fc2

_PROFILE = bool(int(os.environ.get("BANKED_PROFILE", "0")))
_DTYPE = os.environ.get("BANKED_DTYPE", "f16")  # "f16" | "f32r" | "bf16" | "f32"

last_exec_time_ns = None
last_results = None


def _install_ntff_hook():
    """bass_utils' axon trace path imports antenv.axon_hooks, which this image
    lacks; shim it and register the ctypes-based NTFF hook."""
    if "antenv.axon_hooks" in sys.modules:
        return
    mod = types.ModuleType("antenv.axon_hooks")
    mod._hook = None
    mod.set_axon_ntff_profile_hook = lambda h: setattr(mod, "_hook", h)
    mod.get_axon_ntff_profile_hook = lambda: mod._hook
    sys.modules["antenv.axon_hooks"] = mod
    try:
        from trn_agent_boot.trn_boot import _ntff_profile_via_ctypes

        mod.set_axon_ntff_profile_hook(
            _ntff_profile_via_ctypes("/opt/axon/libaxon_pjrt.so")
        )
    except Exception as e:  # profiling is best-effort
        print("ntff hook setup failed:", e)


def _router(tensor_f32, Wr, br):
    """Return (topv, topi) exactly as the reference computes them (jax on CPU)."""
    try:
        import jax
        import jax.numpy as jnp

        cpu = jax.devices("cpu")[0]
        with jax.default_device(cpu):
            t = jax.device_put(jnp.asarray(tensor_f32), cpu)
            w = jax.device_put(jnp.asarray(Wr), cpu)
            b = jax.device_put(jnp.asarray(br), cpu)
            logits = jnp.einsum("bsd,de->bse", t, w) + b
            probs = jax.nn.softmax(logits, axis=-1)
            topv, topi = jax.lax.top_k(probs, NUM_SELECTED)
        return np.asarray(topv), np.asarray(topi)
    except Exception:
        # numpy fallback replicating jax semantics (stable ties by lower index)
        logits = (
            tensor_f32.reshape(-1, D_MODEL) @ np.asarray(Wr, np.float32)
        ) + np.asarray(br, np.float32)
        logits = logits.reshape(tensor_f32.shape[0], tensor_f32.shape[1], NUM_BANKS)
        m = logits.max(axis=-1, keepdims=True)
        e = np.exp(logits - m)
        probs = e / e.sum(axis=-1, keepdims=True)
        order = np.argsort(-probs, axis=-1, kind="stable")
        topi = order[..., :NUM_SELECTED]
        topv = np.take_along_axis(probs, topi, axis=-1)
        return topv.astype(np.float32), topi.astype(np.int32)


def _cap_for(n):
    """Slot capacity for a max bank load of n tokens (even, >= 128)."""
    return max((int(n) + 1) & ~1, P)


def _chunks_for(C):
    """Split C into matmul free-dim chunks <= 512 of near-equal size
    (each >= 256 where possible — float32r needs that for full rate)."""
    out = []
    t0 = 0
    n = (C + 511) // 512
    base = C // n
    rem = C - base * n
    for i in range(n):
        tn = base + (1 if i < rem else 0)
        out.append((t0, tn))
        t0 += tn
    return out


def _build_program(caps, chunks_list, dtype_tag):
    import concourse.mybir as mybir
    import concourse.tile as tile
    from concourse import bacc

    if dtype_tag == "bf16":
        wdt = mybir.dt.bfloat16
    elif dtype_tag == "f16":
        wdt = mybir.dt.float16
    elif dtype_tag == "f32":
        wdt = mybir.dt.float32
    else:
        # float32r end-to-end: walrus requires f32r matmul operands to be
        # produced as f32r, so the dram tensors and sbuf tiles are all f32r
        # (np-side arrays stay float32 — same 4-byte storage).
        wdt = mybir.dt.float32r

    f32 = mybir.dt.float32
    nc = bacc.Bacc("TRN2", target_bir_lowering=False, debug=False, num_devices=N_CORES)

    xts = [
        nc.dram_tensor(f"xt{s}", [KD, P, caps[s]], wdt, kind="ExternalInput").ap()
        for s in range(2)
    ]
    w1 = nc.dram_tensor("w1", [2, MH, P, KD, P], wdt, kind="ExternalInput").ap()
    b1 = nc.dram_tensor("b1", [2, P, MH], f32, kind="ExternalInput").ap()
    w2 = nc.dram_tensor("w2", [2, MO, P, MH, P], wdt, kind="ExternalInput").ap()
    b2 = nc.dram_tensor("b2", [2, P, MO], f32, kind="ExternalInput").ap()
    yts = [
        nc.dram_tensor(f"yt{s}", [MO, P, caps[s]], f32, kind="ExternalOutput").ap()
        for s in range(2)
    ]

    Add = mybir.AluOpType.add
    Max = mybir.AluOpType.max
    tcmax = max(tn for chunks in chunks_list for _, tn in chunks)

    with tile.TileContext(nc) as tc:
        with (
            tc.tile_pool(name="xp", bufs=1) as xp,
            tc.tile_pool(name="bp", bufs=1) as bp,
            tc.tile_pool(name="w1p", bufs=12) as w1p,
            tc.tile_pool(name="w2p", bufs=5) as w2p,
            tc.tile_pool(name="htp", bufs=1) as htp,
            tc.tile_pool(name="yp", bufs=4) as yp,
            tc.tile_pool(name="ps1", bufs=4, space="PSUM") as ps1,
            tc.tile_pool(name="ps2", bufs=4, space="PSUM") as ps2,
        ):
            # Per-(slot, k, chunk) x tiles: contiguous DMAs with fine-grained
            # deps so the first matmuls start as soon as their slice lands.
            # DMA issue is spread over engine queues: w1 on sync, w2 + slot-0 x
            # on scalar, slot-1 x + biases on gpsimd (only SP/ACT/POOL can
            # issue DMAs; gpsimd descriptor issue is the slowest at ~790ns).
            xsb = {}

            def load_x(s, eng):
                # contiguous per-k DMAs with fine-grained deps
                for ci, (t0, tn) in enumerate(chunks_list[s]):
                    for k in range(KD):
                        t = xp.tile(
                            [P, tn], wdt, tag=f"x_{s}_{k}_{ci}", name=f"x_{s}_{k}_{ci}"
                        )
                        eng.dma_start(t[:], xts[s][k, :, t0 : t0 + tn])
                        xsb[(s, k, ci)] = t

            load_x(0, nc.scalar)
            b1sb = bp.tile([P, 2, MH], f32, tag="b1")
            nc.gpsimd.dma_start(b1sb[:], b1.rearrange("e p m -> p e m"))
            b2sb = bp.tile([P, 2, MO], f32, tag="b2")
            nc.gpsimd.dma_start(b2sb[:], b2.rearrange("e p m -> p e m"))

            for s in range(2):
                chunks = chunks_list[s]
                C = caps[s]
                if s == 1:
                    load_x(1, nc.gpsimd)
                ht = htp.tile([P, MH, C], wdt, tag="ht")
                # At kernel start only chunk 0's x is likely resident; emit the
                # first G hm-groups chunk-0-only so the PE has a runway while
                # the rest of x streams in.
                if s == 0 and len(chunks) > 1:
                    G = min(8, MH)
                    group_order = (
                        [(hm, 0) for hm in range(G)]
                        + [(hm, ci) for hm in range(G) for ci in range(1, len(chunks))]
                        + [(hm, ci) for hm in range(G, MH) for ci in range(len(chunks))]
                    )
                else:
                    group_order = [
                        (hm, ci) for hm in range(MH) for ci in range(len(chunks))
                    ]
                w1sbs = {}
                for hm, ci in group_order:
                    if hm not in w1sbs:
                        w1sb = w1p.tile([P, KD, P], wdt, tag="w1")
                        nc.sync.dma_start(w1sb[:], w1[s, hm])
                        w1sbs[hm] = w1sb
                    w1sb = w1sbs[hm]
                    t0, tn = chunks[ci]
                    ps = ps1.tile([P, tcmax], f32, tag="ps1", name=f"ps1_{hm}_{ci}")
                    for k in range(KD):
                        nc.tensor.matmul(
                            ps[:, :tn],
                            w1sb[:, k],
                            xsb[(s, k, ci)][:, :tn],
                            start=(k == 0),
                            stop=(k == KD - 1),
                        )
                    # relu(psum + b1) on DVE (ACT's per-op overhead is huge)
                    nc.vector.tensor_scalar(
                        ht[:, hm, t0 : t0 + tn],
                        ps[:, :tn],
                        b1sb[:, s, hm : hm + 1],
                        0.0,
                        Add,
                        Max,
                    )
                for mo in range(MO):
                    # two half-tiles so the first 16 k2 matmuls can start
                    # after half the weight DMA
                    MHH = MH // 2
                    w2sbs = []
                    for h in range(2):
                        w2sb = w2p.tile([P, MHH, P], wdt, tag="w2", name=f"w2_{h}")
                        nc.scalar.dma_start(
                            w2sb[:], w2[s, mo, :, h * MHH : (h + 1) * MHH]
                        )
                        w2sbs.append(w2sb)
                    pss = [
                        ps2.tile([P, tcmax], f32, tag="ps2", name=f"ps2_{ci}")
                        for ci in range(len(chunks))
                    ]
                    for k2 in range(MH):
                        for ci, (t0, tn) in enumerate(chunks):
                            nc.tensor.matmul(
                                pss[ci][:, :tn],
                                w2sbs[k2 // MHH][:, k2 % MHH],
                                ht[:, k2, t0 : t0 + tn],
                                start=(k2 == 0),
                                stop=(k2 == MH - 1),
                            )
                    for ci, (t0, tn) in enumerate(chunks):
                        ysb = yp.tile([P, tcmax], f32, tag="y")
                        nc.vector.tensor_scalar_add(
                            ysb[:, :tn],
                            pss[ci][:, :tn],
                            b2sb[:, s, mo : mo + 1],
                        )
                        nc.sync.dma_start(yts[s][mo, :, t0 : t0 + tn], ysb[:, :tn])

    nc.compile()
    return nc


def kernel(tensor, Wr, br, W1, b1, W2, b2):
    global last_exec_time_ns, last_results
    from concourse import bass_utils

    t_np = np.asarray(tensor, np.float32)
    B, S, _ = t_np.shape
    T = B * S
    x = np.ascontiguousarray(t_np.reshape(T, D_MODEL))

    topv, topi = _router(t_np, np.asarray(Wr, np.float32), np.asarray(br, np.float32))
    topv = topv.reshape(T, NUM_SELECTED)
    topi = topi.reshape(T, NUM_SELECTED)

    # Per-bank token index lists + gates
    idx = []
    gates = []
    for e in range(NUM_BANKS):
        sel = np.nonzero((topi == e).any(axis=1))[0]
        idx.append(sel)
        g = np.where(topi[sel, 0] == e, topv[sel, 0], topv[sel, 1])
        gates.append(g.astype(np.float32))

    counts = np.array([len(i) for i in idx])
    # Slot 0 hosts the 8 busiest banks, slot 1 the 8 least busy; the two
    # slots get independent capacities, minimizing padded tokens.
    order = np.argsort(-counts, kind="stable")
    slot_banks = [order[:N_CORES], order[N_CORES:]]
    caps = [_cap_for(counts[sb].max()) for sb in slot_banks]
    chunks_list = [_chunks_for(c) for c in caps]

    if _DTYPE == "bf16":
        import ml_dtypes

        np_wdt = ml_dtypes.bfloat16
    elif _DTYPE == "f16":
        np_wdt = np.float16
    else:
        np_wdt = np.float32

    # Feature-major gathered tokens per slot: xt[s][c, k, p, t]
    xt_slot = [np.zeros((N_CORES, KD, P, caps[s]), dtype=np_wdt) for s in range(2)]
    for s in range(2):
        for c in range(N_CORES):
            e = slot_banks[s][c]
            n_e = counts[e]
            if n_e:
                xe_t = x[idx[e]].T.astype(np_wdt)  # [D_MODEL, n_e]
                xt_slot[s][c, :, :, :n_e] = xe_t.reshape(KD, P, n_e)

    W1_np = np.asarray(W1, np.float32)
    W2_np = np.asarray(W2, np.float32)
    w1d = np.ascontiguousarray(
        W1_np.reshape(NUM_BANKS, KD, P, MH, P).transpose(0, 3, 2, 1, 4).astype(np_wdt)
    )
    w2d = np.ascontiguousarray(
        W2_np.reshape(NUM_BANKS, MH, P, MO, P).transpose(0, 3, 2, 1, 4).astype(np_wdt)
    )
    b1d = np.ascontiguousarray(
        np.asarray(b1, np.float32).reshape(NUM_BANKS, MH, P).transpose(0, 2, 1)
    )
    b2d = np.ascontiguousarray(
        np.asarray(b2, np.float32).reshape(NUM_BANKS, MO, P).transpose(0, 2, 1)
    )

    nc = _build_program(caps, chunks_list, _DTYPE)

    in_maps = []
    for c in range(N_CORES):
        pair = [slot_banks[0][c], slot_banks[1][c]]
        in_maps.append(
            {
                "xt0": xt_slot[0][c],
                "xt1": xt_slot[1][c],
                "w1": w1d[pair],
                "b1": b1d[pair],
                "w2": w2d[pair],
                "b2": b2d[pair],
            }
        )

    if _PROFILE:
        _install_ntff_hook()
    res = bass_utils.run_bass_kernel_spmd(
        nc, in_maps, core_ids=list(range(N_CORES)), trace=_PROFILE
    )
    last_exec_time_ns = res.exec_time_ns
    last_results = res

    out = np.zeros((T, D_MODEL), dtype=np.float32)
    for c in range(N_CORES):
        for s in range(2):
            e = slot_banks[s][c]
            n_e = counts[e]
            if n_e:
                ytc = res.results[c][f"yt{s}"]  # [MO, P, caps[s]] f32
                ye = ytc.reshape(D_MODEL, caps[s])[:, :n_e]
                out[idx[e]] += gates[e][:, None] * ye.T
    return out.reshape(B, S, D_MODEL)
